# revision 42
# baseline (speedup 1.0000x reference)
"""Trainium2 Bass kernel for the LGSA block (XCiT-style channel attention +
conv-gated value + MLP with depthwise conv).

Sharding: pure data parallel over batch B=32 across 8 cores (4 images/core).

Per-core design (per image):
  - Activations in [C, N] layout: channels on SBUF partitions (3 tiles of
    128), N = H*W = 1024 on the free axis.  Branch activations (everything
    whose error is suppressed by gamma=1e-6) are fp8e4m3 with host-folded
    power-of-two scales; fp8 DoubleRow matmuls pack two 128-channel k-tiles
    (or two conv taps) per pass.
  - qkT in [N, 768] layout (one fp8 tile, 8 n-subtiles) with per-head
    column interleaving [q_h | k_h] so one Gram matmul per head yields
    q@k^T plus ||q||^2, ||k||^2 on the diagonal; the fp8 scale cancels in
    the l2 normalization.
  - LayerNorm stats via ones-vector matmuls on the tensor engine; LN
    gamma/beta folded into downstream weights on the host.
  - Depthwise 3x3 convs in a flat zero-guarded padded layout: on the
    tensor engine as diagonal matmuls (DoubleRow: two taps per pass), or
    on the vector engine as 9 flat-shifted MACs.
  - The torch-faithful residual-1 scramble (swapaxes(1,2).reshape) via a
    DRAM round trip with a DMA-accumulate onto the residual.
"""

import os
import numpy as np
import ml_dtypes
from contextlib import ExitStack

DBG_STOP = os.environ.get("KBG_STOP", "")

import concourse.bass as bass
import concourse.bacc as bacc
import concourse.mybir as mybir
import concourse.tile as tile
from concourse.bass_utils import run_bass_kernel_spmd

F32 = mybir.dt.float32
BF16 = mybir.dt.bfloat16
F8E4 = mybir.dt.float8e4
AX = mybir.AxisListType
ALU = mybir.AluOpType
ACTF = mybir.ActivationFunctionType
DR = mybir.MatmulPerfMode.DoubleRow

B, C, H, W = 32, 384, 32, 32
N = H * W
NH, DH = 6, 64
HID = 2304
NCORES = 8
CT = C // 128              # 3 channel tiles
HT = HID // 128            # 18 hidden tiles
NT = N // 128              # 8 n tiles
PE_HID = 16                # hid tiles whose dwconv runs on PE (rest on DVE)
EPS_LN = 1e-6
EPS_BN = 1e-5

# fp8 scales (powers of two; folded on host / in activation scales)
S_QK = 32.0                # wqk scale; cancels in l2norm
S_VC = 16.0                # vcw scale; v and xo carry it, folded into g1
S_W = 64.0                 # w1 / w2 / projw / conv-tap scale

TAPS = [(0, 0)] + [(dy, dx) for dy in (-1, 0, 1) for dx in (-1, 0, 1)
                   if (dy, dx) != (0, 0)]
# DoubleRow tap pairs (indices into TAPS): 4 pairs + the center single
TAP_PAIRS = ((1, 2), (3, 4), (5, 6), (7, 8))

# MLP kt pairs for the DoubleRow w2 accumulation; PE-conv and DVE-conv
# tiles are interleaved inside each of the leading pairs.
_rem = list(range(HT - PE_HID, PE_HID))
KT_PAIRS = list(zip(range(HT - PE_HID), range(PE_HID, HT))) + \
    [(_rem[i], _rem[i + 1]) for i in range(0, len(_rem), 2)]

# padded flat conv geometry
PW = W + 2                 # 34
MOFF = 36
MROWS = 18                 # MLP half window rows
MLP_L = MROWS * PW         # 612
MSRC_SZ = MOFF + MLP_L + MOFF
DROWS = H + 2
DW1_L = DROWS * PW
DSRC_SZ = MOFF + DW1_L + MOFF

np_bf16 = ml_dtypes.bfloat16
np_f8 = ml_dtypes.float8_e4m3fn


def _f8(a):
    return np.clip(a, -240.0, 240.0).astype(np_f8)


# ----------------------------------------------------------------------------
# Host-side precompute
# ----------------------------------------------------------------------------

def _pos_embed_host(pos_w, pos_b):
    HID_PE = 32
    scale = 2 * np.pi
    eps = 1e-6
    dim_t = 10000.0 ** (2 * (np.arange(HID_PE) // 2).astype(np.float64) / HID_PE)

    def four(e):
        p = e[:, None] / dim_t
        return np.stack([np.sin(p[:, 0::2]), np.cos(p[:, 1::2])], -1).reshape(
            e.shape[0], HID_PE)

    ye = np.arange(1, H + 1, dtype=np.float64) / (H + eps) * scale
    xe = np.arange(1, W + 1, dtype=np.float64) / (W + eps) * scale
    py = np.broadcast_to(four(ye)[:, None, :], (H, W, HID_PE))
    px = np.broadcast_to(four(xe)[None, :, :], (H, W, HID_PE))
    pos = np.concatenate([py, px], -1) @ pos_w.astype(np.float64).T \
        + pos_b.astype(np.float64)
    return pos.transpose(2, 0, 1).reshape(C, N)      # [C, N]


def _kt_major(a):
    """[T*128, X] -> [128, T*X] with the row tiles side by side."""
    t = a.shape[0] // 128
    return a.reshape(t, 128, a.shape[1]).transpose(1, 0, 2).reshape(
        128, t * a.shape[1])


def _diag_pairs(taps_cn):
    """taps_cn: [n_tiles*128, 9].  Returns [n_tiles, 128, 4*256+128]:
    4 DoubleRow tap-pair diagonal blocks [2,128] plus the center single."""
    ch = taps_cn.shape[0]
    nt = ch // 128
    out = np.zeros((nt, 128, 9 * 128), np.float64)
    idx = np.arange(128)
    for t in range(nt):
        for pi, (ta, tb) in enumerate(TAP_PAIRS):
            out[t, idx, pi * 256 + idx] = taps_cn[t * 128:(t + 1) * 128, ta]
            out[t, idx, pi * 256 + 128 + idx] = taps_cn[t * 128:(t + 1) * 128, tb]
        out[t, idx, 4 * 256 + idx] = taps_cn[t * 128:(t + 1) * 128, 0]
    return out


def _valid_tap_sum(w33):
    ch = w33.shape[0]
    m = np.zeros((ch, H, W), np.float64)
    for dy in (-1, 0, 1):
        for dx in (-1, 0, 1):
            ys = slice(max(0, -dy), H - max(0, dy))
            xs = slice(max(0, -dx), W - max(0, dx))
            m[:, ys, xs] += w33[:, dy + 1, dx + 1][:, None, None]
    return m.reshape(ch, N)


def _host_consts(inp):
    g = {k: np.asarray(v, np.float64) for k, v in inp.items()}
    c = {}

    ln1w, ln1b = g["ln1_w"], g["ln1_b"]
    ln2w, ln2b = g["ln2_w"], g["ln2_b"]

    c["pos"] = _pos_embed_host(g["pos_w"], g["pos_b"]).astype(np_bf16)  # [C,N]

    # qk packed weights [C, 768]: per head [q(64) | k(64)], LN1 affine folded
    Wq = ln1w[:, None] * g["q_w"].T      # [cin, cout]
    Wk = ln1w[:, None] * g["k_w"].T
    bq = g["q_b"] + g["q_w"] @ ln1b
    bk = g["k_b"] + g["k_w"] @ ln1b
    wqk = np.zeros((C, 2 * C), np.float64)
    bqk = np.zeros((2 * C,), np.float64)
    for h in range(NH):
        wqk[:, h * 128:h * 128 + 64] = Wq[:, h * 64:(h + 1) * 64]
        wqk[:, h * 128 + 64:h * 128 + 128] = Wk[:, h * 64:(h + 1) * 64]
        bqk[h * 128:h * 128 + 64] = bq[h * 64:(h + 1) * 64]
        bqk[h * 128 + 64:h * 128 + 128] = bk[h * 64:(h + 1) * 64]
    c["wqk"] = _f8(_kt_major(S_QK * wqk))                      # [128,3*768]
    c["bqk"] = (S_QK * bqk)[None, :].astype(np_bf16)           # [1,768]

    c["vcw"] = _f8(_kt_major(S_VC * ln1w[:, None] * g["vc_w"].T))
    c["bvc"] = (S_VC * (g["vc_b"] + g["vc_w"] @ ln1b)).reshape(
        CT, 128).T.copy().astype(np.float32)                   # [128,CT]

    # dwconv1: LN gamma and BN scale folded into taps; zsh folds the
    # beta border effect + conv bias + BN shift.
    s1 = g["bn_g"] / np.sqrt(g["bn_var"] + EPS_BN)
    w1raw = g["dw_w"][:, 0]                                    # [C,3,3]
    taps1 = np.stack([w1raw[:, dy + 1, dx + 1] for (dy, dx) in TAPS], -1)
    c["dw1d"] = _f8(_diag_pairs(S_W * taps1 * (ln1w * s1)[:, None]))
    zsh1 = (ln1b[:, None] * _valid_tap_sum(w1raw) + g["dw_b"][:, None]) \
        * s1[:, None] + (g["bn_b"] - g["bn_mean"] * s1)[:, None]
    c["zsh1"] = zsh1.astype(np_bf16)                           # [C,N]

    c["projw"] = _f8(_kt_major(S_W * g["proj_w"].T))           # [128,3*384]
    sp = 1.0 / (S_W * S_VC)
    c["g1"] = (sp * g["gamma1"]).reshape(CT, 128).T.copy().astype(np.float32)
    c["g1pb"] = (g["gamma1"] * g["proj_b"]).reshape(CT, 128).T.copy().astype(
        np.float32)                                            # [128,CT]

    c["w1"] = _f8(_kt_major(S_W * ln2w[:, None] * g["mlp_w1"].T))
    c["b1"] = (g["mlp_b1"] + g["mlp_w1"] @ ln2b).reshape(HT, 128).T.copy().astype(
        np.float32)                                            # [128,HT]

    w2raw = g["mlp_dw"][:, 0]                                  # [HID,3,3]
    taps2 = np.stack([w2raw[:, dy + 1, dx + 1] for (dy, dx) in TAPS], -1)
    c["dw2d"] = _f8(_diag_pairs(S_W * taps2[:PE_HID * 128]))
    if HT > PE_HID:
        tt = taps2[PE_HID * 128:].reshape(HT - PE_HID, 128, 9).transpose(1, 0, 2)
        c["dw2t"] = tt.copy().astype(np.float32)               # [128,HT-PE_HID,9]
    c["db2"] = g["mlp_db"].reshape(HT, 128).T.copy().astype(np.float32)

    # w2 in KT_PAIRS order for DoubleRow accumulation
    w2t = (S_W * g["mlp_w2"].T).reshape(HT, 128, C)            # [kt,128,C]
    w2p = np.zeros((128, len(KT_PAIRS) * 2 * C), np.float64)
    for pi, (ka, kb) in enumerate(KT_PAIRS):
        w2p[:, (2 * pi) * C:(2 * pi + 1) * C] = w2t[ka]
        w2p[:, (2 * pi + 1) * C:(2 * pi + 2) * C] = w2t[kb]
    c["w2"] = _f8(w2p)                                         # [128,18*384]
    c["b2row"] = (S_W * g["mlp_b2"])[None, :].astype(np_bf16)  # [1,C]
    c["g2"] = (g["gamma2"] / S_W).reshape(CT, 128).T.copy().astype(np.float32)

    c["temp6"] = np.asarray(inp["temp"], np.float32).reshape(1, NH)

    idn = np.eye(128)
    c["idn"] = idn.astype(np_bf16)
    c["mask6"] = np.tile(idn, (1, NH)).astype(np_bf16)         # [128,768]
    c["ones_col"] = np.ones((128, 1), np_bf16)
    c["ones_1x128"] = np.ones((1, 128), np_bf16)
    c["ones_row"] = np.ones((1, 512), np_bf16)
    c["one11"] = np.ones((1, 1), np_bf16)
    return c


# ----------------------------------------------------------------------------
# Device program
# ----------------------------------------------------------------------------

def _np_to_dt(a):
    if a.dtype == np.float32:
        return F32
    if a.dtype == np_f8:
        return F8E4
    return BF16


def _build_program(cspecs, n_img):
    nc = bacc.Bacc("TRN2", target_bir_lowering=False, debug=False,
                   num_devices=NCORES)
    x_in = nc.declare_dram_parameter("x", [n_img, C, H, W], F32, isOutput=False)
    y_out = nc.declare_dram_parameter("y", [n_img, C, H, W], F32, isOutput=True)
    cin = {k: nc.declare_dram_parameter(k, shape, dt, isOutput=False)
           for k, (shape, dt) in cspecs.items()}

    xv = x_in.rearrange("b (t p) h w -> b t p (h w)", p=128)   # [n_img,CT,128,N]
    yv = y_out.rearrange("b (t p) h w -> b t p (h w)", p=128)

    with tile.TileContext(nc) as tc:
        with ExitStack() as ctx:
            _emit(ctx, tc, nc, xv, yv, cin, n_img)
    nc.compile()
    return nc


def _apv(t, off, dims):
    """Raw AP view into tile t at element offset `off` with [stride, n] dims
    (partition dim inherited)."""
    return bass.AP(tensor=t.tensor, offset=t.offset + off,
                   ap=[t.ap[0]] + [list(d) for d in dims])


def _emit(ctx, tc, nc, xv, yv, cin, n_img):
    ep = ctx.enter_context

    const = ep(tc.tile_pool(name="const", bufs=1))
    sb = {}
    # constants loaded as single tiles
    for k in ("pos", "zsh1"):
        t = cin[k]
        sb[k] = []
        for j in range(t.shape[0] // 128):
            s = const.tile([128, t.shape[1]], t.dtype, tag=f"c_{k}{j}",
                           name=f"c_{k}{j}")
            nc.sync.dma_start(s, t[j * 128:(j + 1) * 128, :])
            sb[k].append(s)
    for k in ("dw1d",):
        t = cin[k]
        sb[k] = []
        for j in range(t.shape[0]):
            s = const.tile([128, t.shape[2]], t.dtype, tag=f"c_{k}{j}",
                           name=f"c_{k}{j}")
            nc.sync.dma_start(s, t[j])
            sb[k].append(s)
    for k in ("wqk", "vcw", "bvc", "g1", "g1pb", "b1",
              "db2", "g2", "idn", "mask6", "ones_col", "dw2t", "bqk", "b2row",
              "ones_1x128", "ones_row", "one11", "temp6"):
        if k not in cin:
            continue
        t = cin[k]
        s = const.tile(list(t.shape), t.dtype, tag=f"c_{k}", name=f"c_{k}")
        nc.sync.dma_start(s, t[:])
        sb[k] = s

    def emit_late_consts():
        # heavy weights not needed until mid-image-0: keep them out of the
        # DMA queue ahead of image 0's x load
        for k in ("projw", "w1", "w2"):
            t = cin[k]
            s = const.tile(list(t.shape), t.dtype, tag=f"c_{k}", name=f"c_{k}")
            nc.sync.dma_start(s, t[:])
            sb[k] = s
        t = cin["dw2d"]
        sb["dw2d"] = []
        for j in range(t.shape[0]):
            s = const.tile([128, t.shape[2]], t.dtype, tag=f"c_dw2d{j}",
                           name=f"c_dw2d{j}")
            nc.sync.dma_start(s, t[j])
            sb["dw2d"].append(s)
    temp_b = const.tile([64, NH], F32, tag="temp_b")
    nc.gpsimd.partition_broadcast(temp_b, sb["temp6"])
    epsln = const.tile([128, 1], F32, tag="epsln")
    nc.vector.memset(epsln, EPS_LN)
    scl64 = const.tile([128, 1], F32, tag="scl64")
    nc.vector.memset(scl64, 1.0 / S_W)

    # working pools
    xfp = ep(tc.tile_pool(name="xf", bufs=2))
    lnp = ep(tc.tile_pool(name="ln", bufs=2))
    rows = ep(tc.tile_pool(name="rows", bufs=2))
    xn0p = ep(tc.tile_pool(name="xn0", bufs=2))
    qkp = ep(tc.tile_pool(name="qk", bufs=1))
    att = ep(tc.tile_pool(name="att", bufs=1))
    vp = ep(tc.tile_pool(name="v", bufs=2))
    xop = ep(tc.tile_pool(name="xo", bufs=1))
    xap = ep(tc.tile_pool(name="xa", bufs=2))
    x1p = ep(tc.tile_pool(name="x1", bufs=2))
    mlpp = ep(tc.tile_pool(name="mlp", bufs=2))
    padp = ep(tc.tile_pool(name="pad", bufs=2))
    outp = ep(tc.tile_pool(name="out", bufs=2))
    dram = ep(tc.tile_pool(name="dram", bufs=2, space="DRAM"))

    # PSUM: acc 2x[128,512]=2 banks, qk [128,768]=2, sm [128,384]=1,
    # pm2 3x[128,512]=3 -> 8 banks
    ps_acc = ep(tc.tile_pool(name="ps_acc", bufs=2, space="PSUM"))
    ps_qk = ep(tc.tile_pool(name="ps_qk", bufs=1, space="PSUM"))
    ps_sm = ep(tc.tile_pool(name="ps_sm", bufs=1, space="PSUM"))
    ps_m2 = ep(tc.tile_pool(name="ps_m2", bufs=1, space="PSUM"))

    CH2 = ((0, 512), (512, 512))

    # ------------------------------------------------------------------
    # Pre-zeroed padded fp8 source slots (guards zeroed once per slot).
    def _zero_pad(t, nrows, guard_rows, total):
        L = nrows * PW
        nc.gpsimd.memset(_apv(t, 0, [[1, MOFF + 1]]), 0.0)
        nc.gpsimd.memset(_apv(t, MOFF + PW - 1, [[PW, nrows], [1, 2]]), 0.0)
        for gr in guard_rows:
            nc.gpsimd.memset(_apv(t, MOFF + gr * PW, [[1, PW]]), 0.0)
        nc.gpsimd.memset(_apv(t, MOFF + L - 1, [[1, total - (MOFF + L - 1)]]),
                         0.0)

    for _b in range(2):
        for half in range(2):
            t = padp.tile([128, MSRC_SZ], F8E4, tag=f"msrc{half}",
                          name=f"msrc{half}")
            _zero_pad(t, MROWS, (0,) if half == 0 else (MROWS - 1,), MSRC_SZ)
        t = padp.tile([128, DSRC_SZ], F8E4, tag="dsrc", name="dsrc")
        _zero_pad(t, DROWS, (0, DROWS - 1), DSRC_SZ)

    # ------------------------------------------------------------------
    def layer_norm_rows(src_bf, sq_bf, tag):
        m_row = rows.tile([1, N], BF16, tag="mrow")
        sd = rows.tile([1, N], BF16, tag="sd")
        for (c0, cn) in CH2:
            prow = ps_acc.tile([128, 512], F32, tag="acc")
            for part, src in ((0, src_bf), (32, sq_bf)):
                for kt in range(CT):
                    nc.tensor.matmul(prow[part:part + 1, :],
                                     lhsT=sb["ones_col"],
                                     rhs=src[kt][:, c0:c0 + cn],
                                     start=(kt == 0), stop=(kt == CT - 1))
            nc.vector.tensor_scalar_mul(m_row[:, c0:c0 + cn], prow[0:1, :],
                                        1.0 / C)
            nc.vector.tensor_scalar_mul(sd[:, c0:c0 + cn], prow[32:33, :],
                                        1.0 / C)
        msq = rows.tile([1, N], BF16, tag="msq")
        nc.scalar.activation(msq, m_row, ACTF.Square)
        nc.vector.tensor_sub(sd, sd, msq)          # var, in place
        nc.scalar.activation(sd, sd, ACTF.Sqrt, bias=epsln[0:1, :])
        psd = ps_sm.tile([128, 384], F32, tag="sm")
        for j in range(NT):
            nc.tensor.matmul(psd[:, j:j + 1], lhsT=sd[:, j * 128:(j + 1) * 128],
                             rhs=sb["one11"], start=True, stop=True)
        rcols = rows.tile([128, NT], BF16, tag="rcols")
        with nc.allow_low_precision(reason="bf16 LN rstd is enough"):
            nc.vector.reciprocal(rcols, psd[:, 0:NT])
        r_row = rows.tile([1, N], BF16, tag="rrow")
        for ci, (c0, cn) in enumerate(CH2):
            prr = ps_acc.tile([128, 512], F32, tag="acc")
            for jj in range(4):
                j = ci * 4 + jj
                nc.tensor.matmul(prr[0:1, jj * 128:(jj + 1) * 128],
                                 lhsT=rcols[:, j:j + 1], rhs=sb["idn"],
                                 start=True, stop=True)
            nc.scalar.activation(r_row[:, c0:c0 + cn], prr[0:1, :], ACTF.Copy)
        m_b = lnp.tile([128, N], BF16, tag="mb")
        nc.gpsimd.partition_broadcast(m_b, m_row)
        r_b = lnp.tile([128, N], BF16, tag="rb")
        nc.gpsimd.partition_broadcast(r_b, r_row)
        return m_b, r_b

    def normalize(src_bf, m_b, r_b, tag):
        """-> one fp8 tile [128, CT*N] (kt-major)."""
        xn = xn0p.tile([128, CT * N], F8E4, tag="xn")
        for kt in range(CT):
            t = lnp.tile([128, N], BF16, tag="cen")
            nc.vector.scalar_tensor_tensor(t, src_bf[kt], 1.0, m_b,
                                           op0=ALU.mult, op1=ALU.subtract)
            with nc.allow_low_precision(reason="fp8 branch activations"):
                nc.vector.scalar_tensor_tensor(
                    xn[:, kt * N:(kt + 1) * N], t, 1.0, r_b,
                    op0=ALU.mult, op1=ALU.mult)
        return xn

    def dwconv_pe(pdw, diag_sb, src, row0):
        """Depthwise conv for 16 output rows starting at padded row `row0+1`
        via 4 DoubleRow tap-pair matmuls + 1 single, accumulating in pdw."""
        for pi, (ta, tb) in enumerate(TAP_PAIRS):
            dya, dxa = TAPS[ta]
            dyb, dxb = TAPS[tb]
            offa = MOFF + (row0 + dya + 1) * PW + 1 + dxa
            offb = MOFF + (row0 + dyb + 1) * PW + 1 + dxb
            nc.tensor.matmul(
                pdw,
                lhsT=_apv(diag_sb, pi * 256, [[128, 2], [1, 128]]),
                rhs=_apv(src, offa, [[offb - offa, 2], [PW, 16], [1, W]]),
                start=(pi == 0), stop=False, perf_mode=DR)
        off0 = MOFF + (row0 + 1) * PW + 1
        nc.tensor.matmul(
            pdw, lhsT=_apv(diag_sb, 4 * 256, [[1, 128]]),
            rhs=_apv(src, off0, [[PW, 16], [1, W]]),
            start=False, stop=True)

    def dwconv_dve_flat(dst, src, taps_ap, L):
        nc.vector.tensor_scalar(_apv(dst, 0, [[1, L]]),
                                _apv(src, MOFF, [[1, L]]),
                                taps_ap[:, 0:1], None, op0=ALU.mult)
        dd = _apv(dst, 0, [[1, L]])
        for ti, (dy, dx) in enumerate(TAPS):
            if ti == 0:
                continue
            s = _apv(src, MOFF + dy * PW + dx, [[1, L]])
            nc.vector.scalar_tensor_tensor(dd, s, taps_ap[:, ti:ti + 1], dd,
                                           op0=ALU.mult, op1=ALU.add)

    # ------------------------------------------------------------------
    N_A = 9

    def emit_image(i):
        # ---- load + pos embed ----
        xf, sq = [], []
        for kt in range(CT):
            xr = xfp.tile([128, N], F32, tag="xraw")
            nc.sync.dma_start(xr, xv[i, kt])
            t = xfp.tile([128, N], BF16, tag=f"xf{kt}")
            nc.gpsimd.tensor_tensor(t, xr, sb["pos"][kt], op=ALU.add)
            xf.append(t)
            s = xfp.tile([128, N], BF16, tag=f"xfsq{kt}")
            nc.vector.tensor_mul(s, t, t)
            sq.append(s)
        yield
        m_b, r_b = layer_norm_rows(xf, sq, "A")
        yield
        xn0 = normalize(xf, m_b, r_b, "A")

        # ---- qkT [N, 768] fp8 (x S_QK; cancels in l2norm) ----
        qk = qkp.tile([128, NT * 768], F8E4, tag="qkT")
        for j in range(NT):
            for (c0, cn) in ((0, 512), (512, 256)):
                pq = ps_acc.tile([128, 512], F32, tag="acc")
                nc.tensor.matmul(
                    pq[:, 0:cn],
                    lhsT=_apv(xn0, j * 128, [[N, 2], [1, 128]]),
                    rhs=_apv(sb["wqk"], c0, [[768, 2], [1, cn]]),
                    start=True, stop=False, perf_mode=DR)
                nc.tensor.matmul(
                    pq[:, 0:cn],
                    lhsT=_apv(xn0, 2 * N + j * 128, [[1, 128]]),
                    rhs=_apv(sb["wqk"], 2 * 768 + c0, [[1, cn]]),
                    start=False, stop=False)
                nc.tensor.matmul(pq[:, 0:cn], lhsT=sb["ones_1x128"],
                                 rhs=sb["bqk"][0:1, c0:c0 + cn],
                                 start=False, stop=True)
                with nc.allow_low_precision(reason="fp8 qk is enough"):
                    nc.scalar.activation(qk[:, j * 768 + c0:j * 768 + c0 + cn],
                                         pq[:, 0:cn], ACTF.Copy)
            if j == 3:
                yield

        # ---- Gram per head (PE, DoubleRow over n-tile pairs) ----
        pG = ps_qk.tile([128, NH * 128], F32, tag="qk")
        for h in range(NH):
            for u in range(NT // 2):
                ap = _apv(qk, (2 * u) * 768 + h * 128, [[768, 2], [1, 128]])
                nc.tensor.matmul(pG[:, h * 128:(h + 1) * 128],
                                 lhsT=ap, rhs=ap,
                                 start=(u == 0), stop=(u == NT // 2 - 1),
                                 perf_mode=DR)
        yield

        # ---- vg = SiLU(BN(dwconv1(xn))) ----
        vg = []
        for kt in range(CT):
            vgt = vp.tile([128, N], BF16, tag=f"vg{kt}")
            xp = padp.tile([128, DSRC_SZ], F8E4, tag="dsrc", name="dsrc")
            with nc.allow_low_precision(reason="fp8 branch activations"):
                nc.scalar.activation(
                    _apv(xp, MOFF + PW + 1, [[PW, H], [1, W]]),
                    _apv(xn0, kt * N, [[W, H], [1, W]]), ACTF.Copy)
            for ch in range(2):
                c0 = ch * 512
                pdw = ps_acc.tile([128, 512], F32, tag="acc")
                dwconv_pe(pdw, sb["dw1d"][kt], xp, ch * 16)
                vgz = vp.tile([128, 512], BF16, tag="vgz")
                nc.vector.scalar_tensor_tensor(
                    vgz, pdw, 1.0 / S_W, sb["zsh1"][kt][:, c0:c0 + 512],
                    op0=ALU.mult, op1=ALU.add)
                nc.scalar.activation(vgt[:, c0:c0 + 512], vgz, ACTF.Silu)
            vg.append(vgt)
        yield

        # ---- vc ; v = vc * vg (x S_VC) ----
        v = []
        for mt in range(CT):
            vt = vp.tile([128, N], BF16, tag=f"v{mt}")
            for (c0, cn) in CH2:
                pvc = ps_acc.tile([128, 512], F32, tag="acc")
                nc.tensor.matmul(
                    pvc,
                    lhsT=_apv(sb["vcw"], mt * 128, [[C, 2], [1, 128]]),
                    rhs=_apv(xn0, c0, [[N, 2], [1, cn]]),
                    start=True, stop=False, perf_mode=DR)
                nc.tensor.matmul(
                    pvc,
                    lhsT=_apv(sb["vcw"], 2 * C + mt * 128, [[1, 128]]),
                    rhs=_apv(xn0, 2 * N + c0, [[1, cn]]),
                    start=False, stop=True)
                nc.vector.scalar_tensor_tensor(
                    vt[:, c0:c0 + cn], pvc, sb["bvc"][:, mt:mt + 1],
                    vg[mt][:, c0:c0 + cn], op0=ALU.add, op1=ALU.mult)
            v.append(vt)
        yield

        # ---- softmax over Gram -> attn ----
        dtmp = att.tile([128, NH * 128], BF16, tag="dtmp")
        nc.vector.tensor_mul(dtmp, pG, sb["mask6"])
        diag6 = att.tile([128, NH], F32, tag="diag6")
        nc.vector.reduce_sum(diag6, dtmp.rearrange("p (h d) -> p h d", h=NH),
                             axis=AX.X)
        nrm = att.tile([128, NH], F32, tag="nrm")
        nc.scalar.activation(nrm, diag6, ACTF.Sqrt)
        nc.vector.tensor_scalar_max(nrm, nrm, 1e-12)
        nr = att.tile([128, NH], F32, tag="nr")
        nc.vector.reciprocal(nr, nrm)
        nrb = att.tile([128, NH], BF16, tag="nrb")
        nc.vector.tensor_copy(nrb, nr)
        rqt = att.tile([64, NH], F32, tag="rqt")
        nc.vector.tensor_mul(rqt, nr[0:64, :], temp_b)
        prk = ps_sm.tile([128, 384], F32, tag="sm")
        for h in range(NH):
            nc.tensor.matmul(prk[0:1, h * 64:(h + 1) * 64],
                             lhsT=nrb[64:128, h:h + 1],
                             rhs=sb["idn"][64:128, 64:128],
                             start=True, stop=True)
        rk_row = att.tile([1, NH * 64], BF16, tag="rk_row")
        nc.scalar.activation(rk_row, prk[0:1, 0:NH * 64], ACTF.Copy)
        rk_b = att.tile([64, NH * 64], BF16, tag="rk_b")
        nc.gpsimd.partition_broadcast(rk_b, rk_row)
        apre = att.tile([64, NH * 64], F32, tag="apre")
        for h in range(NH):
            nc.vector.scalar_tensor_tensor(
                apre[:, h * 64:(h + 1) * 64],
                pG[0:64, h * 128 + 64:h * 128 + 128],
                rqt[:, h:h + 1], rk_b[:, h * 64:(h + 1) * 64],
                op0=ALU.mult, op1=ALU.mult)
        mx = att.tile([64, NH], F32, tag="mx")
        nc.vector.reduce_max(mx, apre.rearrange("p (h d) -> p h d", h=NH),
                             axis=AX.X)
        nmx = att.tile([64, NH], F32, tag="nmx")
        nc.vector.tensor_scalar_mul(nmx, mx, -1.0)
        ex = att.tile([64, NH * 64], BF16, tag="ex")
        for h in range(NH):
            nc.scalar.activation(ex[:, h * 64:(h + 1) * 64],
                                 apre[:, h * 64:(h + 1) * 64],
                                 ACTF.Exp, bias=nmx[:, h:h + 1])
        smm = att.tile([64, NH], F32, tag="smm")
        nc.vector.reduce_sum(smm, ex.rearrange("p (h d) -> p h d", h=NH),
                             axis=AX.X)
        rs = att.tile([64, NH], F32, tag="rs")
        nc.vector.reciprocal(rs, smm)
        attn = att.tile([64, NH * 64], BF16, tag="attn")
        for h in range(NH):
            nc.vector.tensor_scalar_mul(attn[:, h * 64:(h + 1) * 64],
                                        ex[:, h * 64:(h + 1) * 64],
                                        rs[:, h:h + 1])
        # transpose each head; pack two heads per 128x128 block-diagonal
        bd = []
        for p in range(CT):
            b_ = att.tile([128, 128], BF16, tag=f"bd{p}")
            nc.vector.memset(b_, 0.0)
            bd.append(b_)
        for h in range(NH):
            pT = ps_sm.tile([128, 768], BF16, tag="sm")
            nc.tensor.transpose(pT[0:64, 0:64], attn[:, h * 64:(h + 1) * 64],
                                sb["idn"][0:64, 0:64])
            o = (h % 2) * 64
            nc.vector.tensor_copy(bd[h // 2][o:o + 64, o:o + 64], pT[0:64, 0:64])
        yield

        # ---- xo = attn @ v (fp8 out, x S_VC) ; proj; gamma1; scramble ----
        xo = xop.tile([128, CT * N], F8E4, tag="xo")
        for p in range(CT):
            for (c0, cn) in CH2:
                pxo = ps_acc.tile([128, 512], F32, tag="acc")
                nc.tensor.matmul(pxo, lhsT=bd[p], rhs=v[p][:, c0:c0 + cn],
                                 start=True, stop=True)
                with nc.allow_low_precision(reason="fp8 branch activations"):
                    nc.scalar.activation(xo[:, p * N + c0:p * N + c0 + cn],
                                         pxo, ACTF.Copy)

        scr = dram.tile([H, C, W], F32, tag="scr")
        scr_w = scr.rearrange("h c w -> c h w")
        scr_r = scr.rearrange("h c w -> (h c) w").rearrange(
            "(r s) w -> r (s w)", s=H)
        for mt in range(CT):
            for ci, (c0, cn) in enumerate(CH2):
                ppr = ps_acc.tile([128, 512], F32, tag="acc")
                nc.tensor.matmul(
                    ppr,
                    lhsT=_apv(sb["projw"], mt * 128, [[C, 2], [1, 128]]),
                    rhs=_apv(xo, c0, [[N, 2], [1, cn]]),
                    start=True, stop=False, perf_mode=DR)
                nc.tensor.matmul(
                    ppr,
                    lhsT=_apv(sb["projw"], 2 * C + mt * 128, [[1, 128]]),
                    rhs=_apv(xo, 2 * N + c0, [[1, cn]]),
                    start=False, stop=True)
                xa = xap.tile([128, 512], F32, tag="xa")
                nc.scalar.activation(xa, ppr, ACTF.Identity,
                                     scale=sb["g1"][:, mt:mt + 1],
                                     bias=sb["g1pb"][:, mt:mt + 1])
                nc.sync.dma_start(
                    scr_w[mt * 128:(mt + 1) * 128, ci * 16:(ci + 1) * 16],
                    xa.rearrange("p (h w) -> p h w", h=16))

        # ---- residual 1: x1 = x + scrambled(xa) ----
        x1 = []
        for mt in range(CT):
            xt = x1p.tile([128, N], F32, tag=f"x1{mt}")
            nc.sync.dma_start(xt, xv[i, mt])
            nc.gpsimd.dma_start(xt, scr_r[mt * 128:(mt + 1) * 128],
                                accum_op=ALU.add)
            x1.append(xt)
        yield
        # ---- LN2 ----
        x1b, x1sq = [], []
        for mt in range(CT):
            tb = xfp.tile([128, N], BF16, tag=f"xf{mt}")
            nc.vector.tensor_copy(tb, x1[mt])
            x1b.append(tb)
            ts_ = xfp.tile([128, N], BF16, tag=f"xfsq{mt}")
            nc.vector.tensor_mul(ts_, tb, tb)
            x1sq.append(ts_)
        m2_b, r2_b = layer_norm_rows(x1b, x1sq, "B")
        xn20 = normalize(x1b, m2_b, r2_b, "B")
        yield

        # ---- MLP in two y-halves; w2 accumulates DoubleRow kt pairs ----
        for half in range(2):
            yo0 = half * 16
            base = max(0, yo0 - 1)
            r_off = 1 if half == 0 else 0
            o0 = yo0 * W
            pm2 = [ps_m2.tile([128, 512], F32, tag=f"pm2_{mt}",
                              name=f"pm2_{mt}") for mt in range(CT)]
            for pi, pair in enumerate(KT_PAIRS):
                h2p = mlpp.tile([128, 1024], F8E4, tag="h2p")
                for jj, kt in enumerate(pair):
                    h1 = padp.tile([128, MSRC_SZ], F8E4, tag=f"msrc{half}",
                                   name=f"msrc{half}")
                    for (r0, rn) in ((0, 16), (16, 1)):
                        pm1 = ps_acc.tile([128, 512], F32, tag="acc")
                        nc.tensor.matmul(
                            pm1[:, 0:rn * W],
                            lhsT=_apv(sb["w1"], kt * 128, [[HID, 2], [1, 128]]),
                            rhs=_apv(xn20, (base + r0) * W,
                                     [[N, 2], [1, rn * W]]),
                            start=True, stop=False, perf_mode=DR)
                        nc.tensor.matmul(
                            pm1[:, 0:rn * W],
                            lhsT=_apv(sb["w1"], 2 * HID + kt * 128, [[1, 128]]),
                            rhs=_apv(xn20, 2 * N + (base + r0) * W,
                                     [[1, rn * W]]),
                            start=False, stop=True)
                        with nc.allow_low_precision(reason="fp8 h1"):
                            nc.scalar.activation(
                                _apv(h1, MOFF + (r_off + r0) * PW + 1,
                                     [[PW, rn], [1, W]]),
                                pm1[:, 0:rn * W],
                                ACTF.Gelu, bias=sb["b1"][:, kt:kt + 1],
                                scale=scl64[:, 0:1])
                    hs = slice(jj * 512, (jj + 1) * 512)
                    if kt < PE_HID:
                        pdw = ps_acc.tile([128, 512], F32, tag="acc")
                        dwconv_pe(pdw, sb["dw2d"][kt], h1, 0)
                        with nc.allow_low_precision(reason="fp8 h2"):
                            nc.scalar.activation(h2p[:, hs], pdw, ACTF.Gelu,
                                                 bias=sb["db2"][:, kt:kt + 1],
                                                 scale=scl64[:, 0:1])
                    else:
                        cv = mlpp.tile([128, MLP_L], BF16, tag="cv")
                        dwconv_dve_flat(cv, h1,
                                        sb["dw2t"][:, kt - PE_HID, :], MLP_L)
                        with nc.allow_low_precision(reason="fp8 h2"):
                            nc.scalar.activation(
                                h2p[:, hs], _apv(cv, PW + 1, [[PW, 16], [1, W]]),
                                ACTF.Gelu, bias=sb["db2"][:, kt:kt + 1])
                for mt in range(CT):
                    nc.tensor.matmul(
                        pm2[mt],
                        lhsT=_apv(sb["w2"], (2 * pi) * C + mt * 128,
                                  [[C, 2], [1, 128]]),
                        rhs=_apv(h2p, 0, [[512, 2], [1, 512]]),
                        start=(pi == 0), stop=False, perf_mode=DR)
                yield
            # bias row via ones rhs, then final residual
            for mt in range(CT):
                nc.tensor.matmul(pm2[mt],
                                 lhsT=sb["b2row"][0:1, mt * 128:(mt + 1) * 128],
                                 rhs=sb["ones_row"][0:1, 0:512],
                                 start=False, stop=True)
                ot = outp.tile([128, 512], F32, tag="ot")
                nc.vector.scalar_tensor_tensor(ot, pm2[mt],
                                               sb["g2"][:, mt:mt + 1],
                                               x1[mt][:, o0:o0 + 512],
                                               op0=ALU.mult, op1=ALU.add)
                nc.sync.dma_start(yv[i, mt][:, o0:o0 + 512], ot)
            yield

    # Software pipeline: interleave image i's MLP emission with image
    # i+1's attention-phase emission so pool-slot rotation (allocation
    # order) lets the scheduler overlap them across engines.
    # Interleave: image i+1's load+LN1-stats chunks are emitted between
    # image i's x1-load and its LN2 so the two LayerNorm cross-engine
    # chains fill each other's stalls; the rest of i+1's attention phase
    # follows image i's MLP.
    gens = [emit_image(i) for i in range(n_img)]
    pos = [0] * n_img

    def adv(j, upto):
        while pos[j] < upto:
            next(gens[j])
            pos[j] += 1

    adv(0, 1)
    emit_late_consts()
    adv(0, 2)
    if n_img > 1:
        adv(1, 1)                # image 1's load overlaps image 0's A phase
    adv(0, N_A - 1)              # c3..c8 of image 0
    for i in range(n_img):
        if i + 1 < n_img:
            adv(i + 1, 2)        # LN1 stats of i+1 fill the LN2 seam
        adv(i, N_A)              # c9: LN2 of i
        cnt = 0
        while True:
            try:
                next(gens[i])
            except StopIteration:
                break
            cnt += 1
            if cnt == 10 and i + 2 < n_img:
                adv(i + 2, 1)    # prefetch i+2's x load mid-MLP (queue-safe)
        if i + 1 < n_img:
            adv(i + 1, N_A - 1)  # c3..c8 of i+1


# ----------------------------------------------------------------------------
# Entry point
# ----------------------------------------------------------------------------

_PROG_CACHE = {}


def kernel(**inputs):
    consts = _host_consts(inputs)
    cspecs = {k: (list(v.shape), _np_to_dt(v)) for k, v in consts.items()}
    x = np.ascontiguousarray(np.asarray(inputs["x"], np.float32))
    n_img = x.shape[0] // NCORES

    key = (n_img,)
    if key not in _PROG_CACHE:
        _PROG_CACHE[key] = _build_program(cspecs, n_img)
    nc = _PROG_CACHE[key]

    in_maps = []
    for ci in range(NCORES):
        m = {"x": x[ci * n_img:(ci + 1) * n_img]}
        m.update(consts)
        in_maps.append(m)
    res = run_bass_kernel_spmd(nc, in_maps, list(range(NCORES)))
    return np.concatenate([r["y"] for r in res.results], axis=0)


# revision 48
# speedup vs baseline: 1.2057x; 1.2057x over previous
"""Trainium2 Bass kernel for the LGSA block (XCiT-style channel attention +
conv-gated value + MLP with depthwise conv).

Sharding: pure data parallel over batch B=32 across 8 cores (4 images/core).

Per-core design (per image):
  - Activations in [C, N] layout: channels on SBUF partitions (3 tiles of
    128), N = H*W = 1024 on the free axis.  Branch activations (everything
    whose error is suppressed by gamma=1e-6) are fp8e4m3 with host-folded
    power-of-two scales; fp8 DoubleRow matmuls pack two 128-channel k-tiles
    (or two conv taps) per pass.
  - qkT in [N, 768] layout (one fp8 tile, 8 n-subtiles) with per-head
    column interleaving [q_h | k_h] so one Gram matmul per head yields
    q@k^T plus ||q||^2, ||k||^2 on the diagonal; the fp8 scale cancels in
    the l2 normalization.
  - LayerNorm stats via ones-vector matmuls on the tensor engine; LN
    gamma/beta folded into downstream weights on the host.
  - Depthwise 3x3 convs in a flat zero-guarded padded layout: on the
    tensor engine as diagonal matmuls (DoubleRow: two taps per pass), or
    on the vector engine as 9 flat-shifted MACs.
  - The torch-faithful residual-1 scramble (swapaxes(1,2).reshape) via a
    DRAM round trip with a DMA-accumulate onto the residual.
"""

import os
import numpy as np
import ml_dtypes
from contextlib import ExitStack

DBG_STOP = os.environ.get("KBG_STOP", "")

import concourse.bass as bass
import concourse.bacc as bacc
import concourse.mybir as mybir
import concourse.tile as tile
from concourse.bass_utils import run_bass_kernel_spmd

F32 = mybir.dt.float32
BF16 = mybir.dt.bfloat16
F8E4 = mybir.dt.float8e4
AX = mybir.AxisListType
ALU = mybir.AluOpType
ACTF = mybir.ActivationFunctionType
DR = mybir.MatmulPerfMode.DoubleRow

B, C, H, W = 32, 384, 32, 32
N = H * W
NH, DH = 6, 64
HID = 2304
NCORES = 8
CT = C // 128              # 3 channel tiles
HT = HID // 128            # 18 hidden tiles
NT = N // 128              # 8 n tiles
PE_HID = 16                # hid tiles whose dwconv runs on PE (rest on DVE)
EPS_LN = 1e-6
EPS_BN = 1e-5

# fp8 scales (powers of two; folded on host / in activation scales)
S_QK = 32.0                # wqk scale; cancels in l2norm
S_VC = 16.0                # vcw scale; v and xo carry it, folded into g1
S_W = 64.0                 # w1 / w2 / projw / conv-tap scale

TAPS = [(0, 0)] + [(dy, dx) for dy in (-1, 0, 1) for dx in (-1, 0, 1)
                   if (dy, dx) != (0, 0)]
# DoubleRow tap pairs (indices into TAPS): 4 pairs + the center single
TAP_PAIRS = ((1, 2), (3, 4), (5, 6), (7, 8))

# MLP kt pairs for the DoubleRow w2 accumulation; PE-conv and DVE-conv
# tiles are interleaved inside each of the leading pairs.
_rem = list(range(HT - PE_HID, PE_HID))
KT_PAIRS = list(zip(range(HT - PE_HID), range(PE_HID, HT))) + \
    [(_rem[i], _rem[i + 1]) for i in range(0, len(_rem), 2)]

# padded flat conv geometry
PW = W + 2                 # 34
MOFF = 36
MROWS = 18                 # MLP half window rows
MLP_L = MROWS * PW         # 612
MSRC_SZ = MOFF + MLP_L + MOFF
DROWS = H + 2
DW1_L = DROWS * PW
DSRC_SZ = MOFF + DW1_L + MOFF

np_bf16 = ml_dtypes.bfloat16
np_f8 = ml_dtypes.float8_e4m3fn


def _f8(a):
    return np.clip(a, -240.0, 240.0).astype(np_f8)


# ----------------------------------------------------------------------------
# Host-side precompute
# ----------------------------------------------------------------------------

def _pos_embed_host(pos_w, pos_b):
    HID_PE = 32
    scale = 2 * np.pi
    eps = 1e-6
    dim_t = 10000.0 ** (2 * (np.arange(HID_PE) // 2).astype(np.float64) / HID_PE)

    def four(e):
        p = e[:, None] / dim_t
        return np.stack([np.sin(p[:, 0::2]), np.cos(p[:, 1::2])], -1).reshape(
            e.shape[0], HID_PE)

    ye = np.arange(1, H + 1, dtype=np.float64) / (H + eps) * scale
    xe = np.arange(1, W + 1, dtype=np.float64) / (W + eps) * scale
    py = np.broadcast_to(four(ye)[:, None, :], (H, W, HID_PE))
    px = np.broadcast_to(four(xe)[None, :, :], (H, W, HID_PE))
    pos = np.concatenate([py, px], -1) @ pos_w.astype(np.float64).T \
        + pos_b.astype(np.float64)
    return pos.transpose(2, 0, 1).reshape(C, N)      # [C, N]


def _kt_major(a):
    """[T*128, X] -> [128, T*X] with the row tiles side by side."""
    t = a.shape[0] // 128
    return a.reshape(t, 128, a.shape[1]).transpose(1, 0, 2).reshape(
        128, t * a.shape[1])


def _diag_pairs(taps_cn):
    """taps_cn: [n_tiles*128, 9].  Returns [n_tiles, 128, 4*256+128]:
    4 DoubleRow tap-pair diagonal blocks [2,128] plus the center single."""
    ch = taps_cn.shape[0]
    nt = ch // 128
    out = np.zeros((nt, 128, 9 * 128), np.float64)
    idx = np.arange(128)
    for t in range(nt):
        for pi, (ta, tb) in enumerate(TAP_PAIRS):
            out[t, idx, pi * 256 + idx] = taps_cn[t * 128:(t + 1) * 128, ta]
            out[t, idx, pi * 256 + 128 + idx] = taps_cn[t * 128:(t + 1) * 128, tb]
        out[t, idx, 4 * 256 + idx] = taps_cn[t * 128:(t + 1) * 128, 0]
    return out


def _valid_tap_sum(w33):
    ch = w33.shape[0]
    m = np.zeros((ch, H, W), np.float64)
    for dy in (-1, 0, 1):
        for dx in (-1, 0, 1):
            ys = slice(max(0, -dy), H - max(0, dy))
            xs = slice(max(0, -dx), W - max(0, dx))
            m[:, ys, xs] += w33[:, dy + 1, dx + 1][:, None, None]
    return m.reshape(ch, N)


def _host_consts(inp):
    g = {k: np.asarray(v, np.float64) for k, v in inp.items()}
    c = {}

    ln1w, ln1b = g["ln1_w"], g["ln1_b"]
    ln2w, ln2b = g["ln2_w"], g["ln2_b"]

    c["pos"] = _pos_embed_host(g["pos_w"], g["pos_b"]).astype(np_bf16)  # [C,N]

    # qk packed weights [C, 768]: per head [q(64) | k(64)], LN1 affine folded
    Wq = ln1w[:, None] * g["q_w"].T      # [cin, cout]
    Wk = ln1w[:, None] * g["k_w"].T
    bq = g["q_b"] + g["q_w"] @ ln1b
    bk = g["k_b"] + g["k_w"] @ ln1b
    wqk = np.zeros((C, 2 * C), np.float64)
    bqk = np.zeros((2 * C,), np.float64)
    for h in range(NH):
        wqk[:, h * 128:h * 128 + 64] = Wq[:, h * 64:(h + 1) * 64]
        wqk[:, h * 128 + 64:h * 128 + 128] = Wk[:, h * 64:(h + 1) * 64]
        bqk[h * 128:h * 128 + 64] = bq[h * 64:(h + 1) * 64]
        bqk[h * 128 + 64:h * 128 + 128] = bk[h * 64:(h + 1) * 64]
    c["wqk"] = _f8(_kt_major(S_QK * wqk))                      # [128,3*768]
    c["bqk"] = (S_QK * bqk)[None, :].astype(np_bf16)           # [1,768]

    c["vcw"] = _f8(_kt_major(S_VC * ln1w[:, None] * g["vc_w"].T))
    c["bvc"] = (S_VC * (g["vc_b"] + g["vc_w"] @ ln1b)).reshape(
        CT, 128).T.copy().astype(np.float32)                   # [128,CT]

    # dwconv1: LN gamma and BN scale folded into taps; zsh folds the
    # beta border effect + conv bias + BN shift.
    s1 = g["bn_g"] / np.sqrt(g["bn_var"] + EPS_BN)
    w1raw = g["dw_w"][:, 0]                                    # [C,3,3]
    taps1 = np.stack([w1raw[:, dy + 1, dx + 1] for (dy, dx) in TAPS], -1)
    c["dw1d"] = _f8(_diag_pairs(S_W * taps1 * (ln1w * s1)[:, None]))
    zsh1 = (ln1b[:, None] * _valid_tap_sum(w1raw) + g["dw_b"][:, None]) \
        * s1[:, None] + (g["bn_b"] - g["bn_mean"] * s1)[:, None]
    c["zsh1"] = zsh1.astype(np_bf16)                           # [C,N]

    c["projw"] = _f8(_kt_major(S_W * g["proj_w"].T))           # [128,3*384]
    sp = 1.0 / (S_W * S_VC)
    c["g1"] = (sp * g["gamma1"]).reshape(CT, 128).T.copy().astype(np.float32)
    c["g1pb"] = (g["gamma1"] * g["proj_b"]).reshape(CT, 128).T.copy().astype(
        np.float32)                                            # [128,CT]

    c["w1"] = _f8(_kt_major(S_W * ln2w[:, None] * g["mlp_w1"].T))
    c["b1"] = (g["mlp_b1"] + g["mlp_w1"] @ ln2b).reshape(HT, 128).T.copy().astype(
        np.float32)                                            # [128,HT]

    w2raw = g["mlp_dw"][:, 0]                                  # [HID,3,3]
    taps2 = np.stack([w2raw[:, dy + 1, dx + 1] for (dy, dx) in TAPS], -1)
    c["dw2d"] = _f8(_diag_pairs(S_W * taps2[:PE_HID * 128]))
    if HT > PE_HID:
        tt = taps2[PE_HID * 128:].reshape(HT - PE_HID, 128, 9).transpose(1, 0, 2)
        c["dw2t"] = tt.copy().astype(np.float32)               # [128,HT-PE_HID,9]
    c["db2"] = g["mlp_db"].reshape(HT, 128).T.copy().astype(np.float32)

    # w2 in KT_PAIRS order for DoubleRow accumulation
    w2t = (S_W * g["mlp_w2"].T).reshape(HT, 128, C)            # [kt,128,C]
    w2p = np.zeros((128, len(KT_PAIRS) * 2 * C), np.float64)
    for pi, (ka, kb) in enumerate(KT_PAIRS):
        w2p[:, (2 * pi) * C:(2 * pi + 1) * C] = w2t[ka]
        w2p[:, (2 * pi + 1) * C:(2 * pi + 2) * C] = w2t[kb]
    c["w2"] = _f8(w2p)                                         # [128,18*384]
    c["b2row"] = (S_W * g["mlp_b2"])[None, :].astype(np_bf16)  # [1,C]
    c["g2"] = (g["gamma2"] / S_W).reshape(CT, 128).T.copy().astype(np.float32)

    c["temp6"] = np.asarray(inp["temp"], np.float32).reshape(1, NH)

    idn = np.eye(128)
    c["idn"] = idn.astype(np_bf16)
    c["mask6"] = np.tile(idn, (1, NH)).astype(np_bf16)         # [128,768]
    c["ones_col"] = np.ones((128, 1), np_bf16)
    c["ones_1x128"] = np.ones((1, 128), np_bf16)
    c["ones_row"] = np.ones((1, 512), np_bf16)
    c["one11"] = np.ones((1, 1), np_bf16)
    return c


# ----------------------------------------------------------------------------
# Device program
# ----------------------------------------------------------------------------

def _np_to_dt(a):
    if a.dtype == np.float32:
        return F32
    if a.dtype == np_f8:
        return F8E4
    return BF16


def _build_program(cspecs, n_img):
    nc = bacc.Bacc("TRN2", target_bir_lowering=False, debug=False,
                   num_devices=NCORES)
    x_in = nc.declare_dram_parameter("x", [n_img, C, H, W], F32, isOutput=False)
    y_out = nc.declare_dram_parameter("y", [n_img, C, H, W], F32, isOutput=True)
    cin = {k: nc.declare_dram_parameter(k, shape, dt, isOutput=False)
           for k, (shape, dt) in cspecs.items()}

    xv = x_in.rearrange("b (t p) h w -> b t p (h w)", p=128)   # [n_img,CT,128,N]
    yv = y_out.rearrange("b (t p) h w -> b t p (h w)", p=128)

    with tile.TileContext(nc) as tc:
        with ExitStack() as ctx:
            _emit(ctx, tc, nc, xv, yv, cin, n_img)
    nc.compile()
    return nc


def _apv(t, off, dims):
    """Raw AP view into tile t at element offset `off` with [stride, n] dims
    (partition dim inherited)."""
    return bass.AP(tensor=t.tensor, offset=t.offset + off,
                   ap=[t.ap[0]] + [list(d) for d in dims])


def _emit(ctx, tc, nc, xv, yv, cin, n_img):
    ep = ctx.enter_context

    const = ep(tc.tile_pool(name="const", bufs=1))
    sb = {}
    # constants loaded as single tiles
    for k in ("pos", "zsh1"):
        t = cin[k]
        sb[k] = []
        for j in range(t.shape[0] // 128):
            s = const.tile([128, t.shape[1]], t.dtype, tag=f"c_{k}{j}",
                           name=f"c_{k}{j}")
            nc.sync.dma_start(s, t[j * 128:(j + 1) * 128, :])
            sb[k].append(s)
    for k in ("dw1d",):
        t = cin[k]
        sb[k] = []
        for j in range(t.shape[0]):
            s = const.tile([128, t.shape[2]], t.dtype, tag=f"c_{k}{j}",
                           name=f"c_{k}{j}")
            nc.sync.dma_start(s, t[j])
            sb[k].append(s)
    for k in ("wqk", "vcw", "bvc", "g1", "g1pb", "b1",
              "db2", "g2", "idn", "mask6", "ones_col", "dw2t", "bqk", "b2row",
              "ones_1x128", "ones_row", "one11", "temp6"):
        if k not in cin:
            continue
        t = cin[k]
        s = const.tile(list(t.shape), t.dtype, tag=f"c_{k}", name=f"c_{k}")
        nc.sync.dma_start(s, t[:])
        sb[k] = s

    def emit_late_consts():
        # heavy weights not needed until mid-image-0: keep them out of the
        # DMA queue ahead of image 0's x load
        for k in ("projw", "w1", "w2"):
            t = cin[k]
            s = const.tile(list(t.shape), t.dtype, tag=f"c_{k}", name=f"c_{k}")
            nc.sync.dma_start(s, t[:])
            sb[k] = s
        t = cin["dw2d"]
        sb["dw2d"] = []
        for j in range(t.shape[0]):
            s = const.tile([128, t.shape[2]], t.dtype, tag=f"c_dw2d{j}",
                           name=f"c_dw2d{j}")
            nc.sync.dma_start(s, t[j])
            sb["dw2d"].append(s)
    temp_b = const.tile([64, NH], F32, tag="temp_b")
    nc.gpsimd.partition_broadcast(temp_b, sb["temp6"])
    epsln = const.tile([128, 1], F32, tag="epsln")
    nc.vector.memset(epsln, EPS_LN)
    scl64 = const.tile([128, 1], F32, tag="scl64")
    nc.vector.memset(scl64, 1.0 / S_W)

    # working pools
    xfp = ep(tc.tile_pool(name="xf", bufs=2))
    lnp = ep(tc.tile_pool(name="ln", bufs=2))
    rows = ep(tc.tile_pool(name="rows", bufs=2))
    xn0p = ep(tc.tile_pool(name="xn0", bufs=2))
    qkp = ep(tc.tile_pool(name="qk", bufs=1))
    att = ep(tc.tile_pool(name="att", bufs=1))
    vp = ep(tc.tile_pool(name="v", bufs=2))
    xop = ep(tc.tile_pool(name="xo", bufs=1))
    xap = ep(tc.tile_pool(name="xa", bufs=2))
    x1p = ep(tc.tile_pool(name="x1", bufs=2))
    mlpp = ep(tc.tile_pool(name="mlp", bufs=2))
    padp = ep(tc.tile_pool(name="pad", bufs=2))
    outp = ep(tc.tile_pool(name="out", bufs=2))
    dram = ep(tc.tile_pool(name="dram", bufs=2, space="DRAM"))

    # PSUM: acc 2x[128,512]=2 banks, qk [128,768]=2, sm [128,384]=1,
    # pm2 3x[128,512]=3 -> 8 banks
    ps_acc = ep(tc.tile_pool(name="ps_acc", bufs=2, space="PSUM"))
    ps_qk = ep(tc.tile_pool(name="ps_qk", bufs=1, space="PSUM"))
    ps_sm = ep(tc.tile_pool(name="ps_sm", bufs=1, space="PSUM"))
    ps_m2 = ep(tc.tile_pool(name="ps_m2", bufs=1, space="PSUM"))

    CH2 = ((0, 512), (512, 512))

    # ------------------------------------------------------------------
    # Pre-zeroed padded fp8 source slots (guards zeroed once per slot).
    def _zero_pad(t, nrows, guard_rows, total):
        L = nrows * PW
        nc.gpsimd.memset(_apv(t, 0, [[1, MOFF + 1]]), 0.0)
        nc.gpsimd.memset(_apv(t, MOFF + PW - 1, [[PW, nrows], [1, 2]]), 0.0)
        for gr in guard_rows:
            nc.gpsimd.memset(_apv(t, MOFF + gr * PW, [[1, PW]]), 0.0)
        nc.gpsimd.memset(_apv(t, MOFF + L - 1, [[1, total - (MOFF + L - 1)]]),
                         0.0)

    for _b in range(2):
        for half in range(2):
            t = padp.tile([128, MSRC_SZ], F8E4, tag=f"msrc{half}",
                          name=f"msrc{half}")
            _zero_pad(t, MROWS, (0,) if half == 0 else (MROWS - 1,), MSRC_SZ)
        t = padp.tile([128, DSRC_SZ], F8E4, tag="dsrc", name="dsrc")
        _zero_pad(t, DROWS, (0, DROWS - 1), DSRC_SZ)

    # ------------------------------------------------------------------
    def layer_norm_rows(src_bf, sq_bf, tag):
        m_row = rows.tile([1, N], BF16, tag="mrow")
        sd = rows.tile([1, N], BF16, tag="sd")
        for (c0, cn) in CH2:
            prow = ps_acc.tile([128, 512], F32, tag="acc")
            for part, src in ((0, src_bf), (32, sq_bf)):
                for kt in range(CT):
                    nc.tensor.matmul(prow[part:part + 1, :],
                                     lhsT=sb["ones_col"],
                                     rhs=src[kt][:, c0:c0 + cn],
                                     start=(kt == 0), stop=(kt == CT - 1))
            nc.vector.tensor_scalar_mul(m_row[:, c0:c0 + cn], prow[0:1, :],
                                        1.0 / C)
            nc.vector.tensor_scalar_mul(sd[:, c0:c0 + cn], prow[32:33, :],
                                        1.0 / C)
        msq = rows.tile([1, N], BF16, tag="msq")
        nc.scalar.activation(msq, m_row, ACTF.Square)
        nc.vector.tensor_sub(sd, sd, msq)          # var, in place
        nc.scalar.activation(sd, sd, ACTF.Sqrt, bias=epsln[0:1, :])
        psd = ps_sm.tile([128, 384], F32, tag="sm")
        for j in range(NT):
            nc.tensor.matmul(psd[:, j:j + 1], lhsT=sd[:, j * 128:(j + 1) * 128],
                             rhs=sb["one11"], start=True, stop=True)
        rcols = rows.tile([128, NT], BF16, tag="rcols")
        with nc.allow_low_precision(reason="bf16 LN rstd is enough"):
            nc.vector.reciprocal(rcols, psd[:, 0:NT])
        r_row = rows.tile([1, N], BF16, tag="rrow")
        for ci, (c0, cn) in enumerate(CH2):
            prr = ps_acc.tile([128, 512], F32, tag="acc")
            for jj in range(4):
                j = ci * 4 + jj
                nc.tensor.matmul(prr[0:1, jj * 128:(jj + 1) * 128],
                                 lhsT=rcols[:, j:j + 1], rhs=sb["idn"],
                                 start=True, stop=True)
            nc.scalar.activation(r_row[:, c0:c0 + cn], prr[0:1, :], ACTF.Copy)
        m_b = lnp.tile([128, N], BF16, tag="mb")
        nc.gpsimd.partition_broadcast(m_b, m_row)
        r_b = lnp.tile([128, N], BF16, tag="rb")
        nc.gpsimd.partition_broadcast(r_b, r_row)
        return m_b, r_b

    def normalize(src_bf, m_b, r_b, tag):
        """-> one fp8 tile [128, CT*N] (kt-major)."""
        xn = xn0p.tile([128, CT * N], F8E4, tag="xn")
        for kt in range(CT):
            t = lnp.tile([128, N], BF16, tag="cen")
            nc.vector.scalar_tensor_tensor(t, src_bf[kt], 1.0, m_b,
                                           op0=ALU.mult, op1=ALU.subtract)
            with nc.allow_low_precision(reason="fp8 branch activations"):
                nc.vector.scalar_tensor_tensor(
                    xn[:, kt * N:(kt + 1) * N], t, 1.0, r_b,
                    op0=ALU.mult, op1=ALU.mult)
        return xn

    def dwconv_pe(pdw, diag_sb, src, row0):
        """Depthwise conv for 16 output rows starting at padded row `row0+1`
        via 4 DoubleRow tap-pair matmuls + 1 single, accumulating in pdw."""
        for pi, (ta, tb) in enumerate(TAP_PAIRS):
            dya, dxa = TAPS[ta]
            dyb, dxb = TAPS[tb]
            offa = MOFF + (row0 + dya + 1) * PW + 1 + dxa
            offb = MOFF + (row0 + dyb + 1) * PW + 1 + dxb
            nc.tensor.matmul(
                pdw,
                lhsT=_apv(diag_sb, pi * 256, [[128, 2], [1, 128]]),
                rhs=_apv(src, offa, [[offb - offa, 2], [PW, 16], [1, W]]),
                start=(pi == 0), stop=False, perf_mode=DR)
        off0 = MOFF + (row0 + 1) * PW + 1
        nc.tensor.matmul(
            pdw, lhsT=_apv(diag_sb, 4 * 256, [[1, 128]]),
            rhs=_apv(src, off0, [[PW, 16], [1, W]]),
            start=False, stop=True)

    def dwconv_dve_flat(dst, src, taps_ap, L):
        nc.vector.tensor_scalar(_apv(dst, 0, [[1, L]]),
                                _apv(src, MOFF, [[1, L]]),
                                taps_ap[:, 0:1], None, op0=ALU.mult)
        dd = _apv(dst, 0, [[1, L]])
        for ti, (dy, dx) in enumerate(TAPS):
            if ti == 0:
                continue
            s = _apv(src, MOFF + dy * PW + dx, [[1, L]])
            nc.vector.scalar_tensor_tensor(dd, s, taps_ap[:, ti:ti + 1], dd,
                                           op0=ALU.mult, op1=ALU.add)

    # ------------------------------------------------------------------
    N_A = 10

    def emit_image(i):
        # ---- load (DMA only; prefetched mid-MLP of image i-2) ----
        xrs = []
        for kt in range(CT):
            xr = xfp.tile([128, N], F32, tag="xraw")
            nc.sync.dma_start(xr, xv[i, kt])
            xrs.append(xr)
        yield
        # ---- pos embed + squares ----
        xf, sq = [], []
        for kt in range(CT):
            t = xfp.tile([128, N], BF16, tag=f"xf{kt}")
            nc.gpsimd.tensor_tensor(t, xrs[kt], sb["pos"][kt], op=ALU.add)
            xf.append(t)
            s = xfp.tile([128, N], BF16, tag=f"xfsq{kt}")
            nc.vector.tensor_mul(s, t, t)
            sq.append(s)
        yield
        m_b, r_b = layer_norm_rows(xf, sq, "A")
        yield
        xn0 = normalize(xf, m_b, r_b, "A")

        # ---- qkT [N, 768] fp8 (x S_QK; cancels in l2norm) ----
        qk = qkp.tile([128, NT * 768], F8E4, tag="qkT")
        for j in range(NT):
            for (c0, cn) in ((0, 512), (512, 256)):
                pq = ps_acc.tile([128, 512], F32, tag="acc")
                nc.tensor.matmul(
                    pq[:, 0:cn],
                    lhsT=_apv(xn0, j * 128, [[N, 2], [1, 128]]),
                    rhs=_apv(sb["wqk"], c0, [[768, 2], [1, cn]]),
                    start=True, stop=False, perf_mode=DR)
                nc.tensor.matmul(
                    pq[:, 0:cn],
                    lhsT=_apv(xn0, 2 * N + j * 128, [[1, 128]]),
                    rhs=_apv(sb["wqk"], 2 * 768 + c0, [[1, cn]]),
                    start=False, stop=False)
                nc.tensor.matmul(pq[:, 0:cn], lhsT=sb["ones_1x128"],
                                 rhs=sb["bqk"][0:1, c0:c0 + cn],
                                 start=False, stop=True)
                with nc.allow_low_precision(reason="fp8 qk is enough"):
                    nc.scalar.activation(qk[:, j * 768 + c0:j * 768 + c0 + cn],
                                         pq[:, 0:cn], ACTF.Copy)
            if j == 3:
                yield

        # ---- Gram per head (PE, DoubleRow over n-tile pairs) ----
        pG = ps_qk.tile([128, NH * 128], F32, tag="qk")
        for h in range(NH):
            for u in range(NT // 2):
                ap = _apv(qk, (2 * u) * 768 + h * 128, [[768, 2], [1, 128]])
                nc.tensor.matmul(pG[:, h * 128:(h + 1) * 128],
                                 lhsT=ap, rhs=ap,
                                 start=(u == 0), stop=(u == NT // 2 - 1),
                                 perf_mode=DR)
        yield

        # ---- vg = SiLU(BN(dwconv1(xn))) ----
        vg = []
        for kt in range(CT):
            vgt = vp.tile([128, N], BF16, tag=f"vg{kt}")
            xp = padp.tile([128, DSRC_SZ], F8E4, tag="dsrc", name="dsrc")
            with nc.allow_low_precision(reason="fp8 branch activations"):
                nc.scalar.activation(
                    _apv(xp, MOFF + PW + 1, [[PW, H], [1, W]]),
                    _apv(xn0, kt * N, [[W, H], [1, W]]), ACTF.Copy)
            for ch in range(2):
                c0 = ch * 512
                pdw = ps_acc.tile([128, 512], F32, tag="acc")
                dwconv_pe(pdw, sb["dw1d"][kt], xp, ch * 16)
                vgz = vp.tile([128, 512], BF16, tag="vgz")
                nc.vector.scalar_tensor_tensor(
                    vgz, pdw, 1.0 / S_W, sb["zsh1"][kt][:, c0:c0 + 512],
                    op0=ALU.mult, op1=ALU.add)
                nc.scalar.activation(vgt[:, c0:c0 + 512], vgz, ACTF.Silu)
            vg.append(vgt)
        yield

        # ---- vc ; v = vc * vg (x S_VC) ----
        v = []
        for mt in range(CT):
            vt = vp.tile([128, N], BF16, tag=f"v{mt}")
            for (c0, cn) in CH2:
                pvc = ps_acc.tile([128, 512], F32, tag="acc")
                nc.tensor.matmul(
                    pvc,
                    lhsT=_apv(sb["vcw"], mt * 128, [[C, 2], [1, 128]]),
                    rhs=_apv(xn0, c0, [[N, 2], [1, cn]]),
                    start=True, stop=False, perf_mode=DR)
                nc.tensor.matmul(
                    pvc,
                    lhsT=_apv(sb["vcw"], 2 * C + mt * 128, [[1, 128]]),
                    rhs=_apv(xn0, 2 * N + c0, [[1, cn]]),
                    start=False, stop=True)
                nc.vector.scalar_tensor_tensor(
                    vt[:, c0:c0 + cn], pvc, sb["bvc"][:, mt:mt + 1],
                    vg[mt][:, c0:c0 + cn], op0=ALU.add, op1=ALU.mult)
            v.append(vt)
        yield

        # ---- softmax over Gram -> attn ----
        dtmp = att.tile([128, NH * 128], BF16, tag="dtmp")
        nc.vector.tensor_mul(dtmp, pG, sb["mask6"])
        diag6 = att.tile([128, NH], F32, tag="diag6")
        nc.vector.reduce_sum(diag6, dtmp.rearrange("p (h d) -> p h d", h=NH),
                             axis=AX.X)
        nrm = att.tile([128, NH], F32, tag="nrm")
        nc.scalar.activation(nrm, diag6, ACTF.Sqrt)
        nc.vector.tensor_scalar_max(nrm, nrm, 1e-12)
        nr = att.tile([128, NH], F32, tag="nr")
        nc.vector.reciprocal(nr, nrm)
        nrb = att.tile([128, NH], BF16, tag="nrb")
        nc.vector.tensor_copy(nrb, nr)
        rqt = att.tile([64, NH], F32, tag="rqt")
        nc.vector.tensor_mul(rqt, nr[0:64, :], temp_b)
        prk = ps_sm.tile([128, 384], F32, tag="sm")
        for h in range(NH):
            nc.tensor.matmul(prk[0:1, h * 64:(h + 1) * 64],
                             lhsT=nrb[64:128, h:h + 1],
                             rhs=sb["idn"][64:128, 64:128],
                             start=True, stop=True)
        rk_row = att.tile([1, NH * 64], BF16, tag="rk_row")
        nc.scalar.activation(rk_row, prk[0:1, 0:NH * 64], ACTF.Copy)
        rk_b = att.tile([64, NH * 64], BF16, tag="rk_b")
        nc.gpsimd.partition_broadcast(rk_b, rk_row)
        apre = att.tile([64, NH * 64], F32, tag="apre")
        for h in range(NH):
            nc.vector.scalar_tensor_tensor(
                apre[:, h * 64:(h + 1) * 64],
                pG[0:64, h * 128 + 64:h * 128 + 128],
                rqt[:, h:h + 1], rk_b[:, h * 64:(h + 1) * 64],
                op0=ALU.mult, op1=ALU.mult)
        mx = att.tile([64, NH], F32, tag="mx")
        nc.vector.reduce_max(mx, apre.rearrange("p (h d) -> p h d", h=NH),
                             axis=AX.X)
        nmx = att.tile([64, NH], F32, tag="nmx")
        nc.vector.tensor_scalar_mul(nmx, mx, -1.0)
        ex = att.tile([64, NH * 64], BF16, tag="ex")
        for h in range(NH):
            nc.scalar.activation(ex[:, h * 64:(h + 1) * 64],
                                 apre[:, h * 64:(h + 1) * 64],
                                 ACTF.Exp, bias=nmx[:, h:h + 1])
        smm = att.tile([64, NH], F32, tag="smm")
        nc.vector.reduce_sum(smm, ex.rearrange("p (h d) -> p h d", h=NH),
                             axis=AX.X)
        rs = att.tile([64, NH], F32, tag="rs")
        nc.vector.reciprocal(rs, smm)
        attn = att.tile([64, NH * 64], BF16, tag="attn")
        for h in range(NH):
            nc.vector.tensor_scalar_mul(attn[:, h * 64:(h + 1) * 64],
                                        ex[:, h * 64:(h + 1) * 64],
                                        rs[:, h:h + 1])
        # transpose each head; pack two heads per 128x128 block-diagonal
        bd = []
        for p in range(CT):
            b_ = att.tile([128, 128], BF16, tag=f"bd{p}")
            nc.vector.memset(b_, 0.0)
            bd.append(b_)
        for h in range(NH):
            pT = ps_sm.tile([128, 768], BF16, tag="sm")
            nc.tensor.transpose(pT[0:64, 0:64], attn[:, h * 64:(h + 1) * 64],
                                sb["idn"][0:64, 0:64])
            o = (h % 2) * 64
            nc.vector.tensor_copy(bd[h // 2][o:o + 64, o:o + 64], pT[0:64, 0:64])
        yield

        # ---- xo = attn @ v (fp8 out, x S_VC) ; proj; gamma1; scramble ----
        xo = xop.tile([128, CT * N], F8E4, tag="xo")
        for p in range(CT):
            for (c0, cn) in CH2:
                pxo = ps_acc.tile([128, 512], F32, tag="acc")
                nc.tensor.matmul(pxo, lhsT=bd[p], rhs=v[p][:, c0:c0 + cn],
                                 start=True, stop=True)
                with nc.allow_low_precision(reason="fp8 branch activations"):
                    nc.scalar.activation(xo[:, p * N + c0:p * N + c0 + cn],
                                         pxo, ACTF.Copy)

        scr = dram.tile([H, C, W], F32, tag="scr")
        scr_w = scr.rearrange("h c w -> c h w")
        scr_r = scr.rearrange("h c w -> (h c) w").rearrange(
            "(r s) w -> r (s w)", s=H)
        for mt in range(CT):
            for ci, (c0, cn) in enumerate(CH2):
                ppr = ps_acc.tile([128, 512], F32, tag="acc")
                nc.tensor.matmul(
                    ppr,
                    lhsT=_apv(sb["projw"], mt * 128, [[C, 2], [1, 128]]),
                    rhs=_apv(xo, c0, [[N, 2], [1, cn]]),
                    start=True, stop=False, perf_mode=DR)
                nc.tensor.matmul(
                    ppr,
                    lhsT=_apv(sb["projw"], 2 * C + mt * 128, [[1, 128]]),
                    rhs=_apv(xo, 2 * N + c0, [[1, cn]]),
                    start=False, stop=True)
                xa = xap.tile([128, 512], F32, tag="xa")
                nc.scalar.activation(xa, ppr, ACTF.Identity,
                                     scale=sb["g1"][:, mt:mt + 1],
                                     bias=sb["g1pb"][:, mt:mt + 1])
                nc.sync.dma_start(
                    scr_w[mt * 128:(mt + 1) * 128, ci * 16:(ci + 1) * 16],
                    xa.rearrange("p (h w) -> p h w", h=16))

        # ---- residual 1: x1 = x + scrambled(xa) ----
        x1 = []
        for mt in range(CT):
            xt = x1p.tile([128, N], F32, tag=f"x1{mt}")
            nc.sync.dma_start(xt, xv[i, mt])
            nc.gpsimd.dma_start(xt, scr_r[mt * 128:(mt + 1) * 128],
                                accum_op=ALU.add)
            x1.append(xt)
        yield
        # ---- LN2 ----
        x1b, x1sq = [], []
        for mt in range(CT):
            tb = xfp.tile([128, N], BF16, tag=f"xf{mt}")
            nc.vector.tensor_copy(tb, x1[mt])
            x1b.append(tb)
            ts_ = xfp.tile([128, N], BF16, tag=f"xfsq{mt}")
            nc.vector.tensor_mul(ts_, tb, tb)
            x1sq.append(ts_)
        m2_b, r2_b = layer_norm_rows(x1b, x1sq, "B")
        xn20 = normalize(x1b, m2_b, r2_b, "B")
        yield

        # ---- MLP in two y-halves; w2 accumulates DoubleRow kt pairs ----
        for half in range(2):
            yo0 = half * 16
            base = max(0, yo0 - 1)
            r_off = 1 if half == 0 else 0
            o0 = yo0 * W
            pm2 = [ps_m2.tile([128, 512], F32, tag=f"pm2_{mt}",
                              name=f"pm2_{mt}") for mt in range(CT)]
            for pi, pair in enumerate(KT_PAIRS):
                h2p = mlpp.tile([128, 1024], F8E4, tag="h2p")
                for jj, kt in enumerate(pair):
                    h1 = padp.tile([128, MSRC_SZ], F8E4, tag=f"msrc{half}",
                                   name=f"msrc{half}")
                    for (r0, rn) in ((0, 16), (16, 1)):
                        pm1 = ps_acc.tile([128, 512], F32, tag="acc")
                        nc.tensor.matmul(
                            pm1[:, 0:rn * W],
                            lhsT=_apv(sb["w1"], kt * 128, [[HID, 2], [1, 128]]),
                            rhs=_apv(xn20, (base + r0) * W,
                                     [[N, 2], [1, rn * W]]),
                            start=True, stop=False, perf_mode=DR)
                        nc.tensor.matmul(
                            pm1[:, 0:rn * W],
                            lhsT=_apv(sb["w1"], 2 * HID + kt * 128, [[1, 128]]),
                            rhs=_apv(xn20, 2 * N + (base + r0) * W,
                                     [[1, rn * W]]),
                            start=False, stop=True)
                        with nc.allow_low_precision(reason="fp8 h1"):
                            nc.scalar.activation(
                                _apv(h1, MOFF + (r_off + r0) * PW + 1,
                                     [[PW, rn], [1, W]]),
                                pm1[:, 0:rn * W],
                                ACTF.Gelu, bias=sb["b1"][:, kt:kt + 1],
                                scale=scl64[:, 0:1])
                    hs = slice(jj * 512, (jj + 1) * 512)
                    if kt < PE_HID:
                        pdw = ps_acc.tile([128, 512], F32, tag="acc")
                        dwconv_pe(pdw, sb["dw2d"][kt], h1, 0)
                        with nc.allow_low_precision(reason="fp8 h2"):
                            nc.scalar.activation(h2p[:, hs], pdw, ACTF.Gelu,
                                                 bias=sb["db2"][:, kt:kt + 1],
                                                 scale=scl64[:, 0:1])
                    else:
                        cv = mlpp.tile([128, MLP_L], BF16, tag="cv")
                        dwconv_dve_flat(cv, h1,
                                        sb["dw2t"][:, kt - PE_HID, :], MLP_L)
                        with nc.allow_low_precision(reason="fp8 h2"):
                            nc.scalar.activation(
                                h2p[:, hs], _apv(cv, PW + 1, [[PW, 16], [1, W]]),
                                ACTF.Gelu, bias=sb["db2"][:, kt:kt + 1])
                for mt in range(CT):
                    nc.tensor.matmul(
                        pm2[mt],
                        lhsT=_apv(sb["w2"], (2 * pi) * C + mt * 128,
                                  [[C, 2], [1, 128]]),
                        rhs=_apv(h2p, 0, [[512, 2], [1, 512]]),
                        start=(pi == 0), stop=False, perf_mode=DR)
                yield
            # bias row via ones rhs, then final residual
            for mt in range(CT):
                nc.tensor.matmul(pm2[mt],
                                 lhsT=sb["b2row"][0:1, mt * 128:(mt + 1) * 128],
                                 rhs=sb["ones_row"][0:1, 0:512],
                                 start=False, stop=True)
                ot = outp.tile([128, 512], F32, tag="ot")
                nc.vector.scalar_tensor_tensor(ot, pm2[mt],
                                               sb["g2"][:, mt:mt + 1],
                                               x1[mt][:, o0:o0 + 512],
                                               op0=ALU.mult, op1=ALU.add)
                nc.sync.dma_start(yv[i, mt][:, o0:o0 + 512], ot)
            yield

    # Software pipeline: interleave image i's MLP emission with image
    # i+1's attention-phase emission so pool-slot rotation (allocation
    # order) lets the scheduler overlap them across engines.
    # Interleave: image i+1's load+LN1-stats chunks are emitted between
    # image i's x1-load and its LN2 so the two LayerNorm cross-engine
    # chains fill each other's stalls; the rest of i+1's attention phase
    # follows image i's MLP.
    gens = [emit_image(i) for i in range(n_img)]
    pos = [0] * n_img

    def adv(j, upto):
        while pos[j] < upto:
            next(gens[j])
            pos[j] += 1

    adv(0, 1)                    # image 0 x-load DMA first
    emit_late_consts()
    if n_img > 1:
        adv(1, 1)                # image 1 x-load overlaps image 0 A phase
    adv(0, N_A - 1)              # image 0 through c8
    for i in range(n_img):
        if i + 1 < n_img:
            adv(i + 1, 3)        # pos/sq + LN1 stats of i+1 fill the seam
        adv(i, N_A)              # LN2 of i
        cnt = 0
        while True:
            try:
                next(gens[i])
            except StopIteration:
                break
            cnt += 1
            if cnt == 10 and i + 2 < n_img:
                adv(i + 2, 1)    # prefetch i+2's x-load DMA mid-MLP
        if i + 1 < n_img:
            adv(i + 1, N_A - 1)  # rest of i+1's attention phase


# ----------------------------------------------------------------------------
# Entry point
# ----------------------------------------------------------------------------

_PROG_CACHE = {}


def kernel(**inputs):
    consts = _host_consts(inputs)
    cspecs = {k: (list(v.shape), _np_to_dt(v)) for k, v in consts.items()}
    x = np.ascontiguousarray(np.asarray(inputs["x"], np.float32))
    n_img = x.shape[0] // NCORES

    key = (n_img,)
    if key not in _PROG_CACHE:
        _PROG_CACHE[key] = _build_program(cspecs, n_img)
    nc = _PROG_CACHE[key]

    in_maps = []
    for ci in range(NCORES):
        m = {"x": x[ci * n_img:(ci + 1) * n_img]}
        m.update(consts)
        in_maps.append(m)
    res = run_bass_kernel_spmd(nc, in_maps, list(range(NCORES)))
    return np.concatenate([r["y"] for r in res.results], axis=0)
